# revision 7
# baseline (speedup 1.0000x reference)
"""KoLeo loss kernel for Trainium2 (8 NeuronCores).

Computes -mean(log(||x_i - x_{nn(i)} + eps||)) where x = row-normalized
student_output and nn(i) is the nearest neighbor by max inner product
(diagonal excluded).

For unit vectors ||x_i - x_j||^2 = 2 - 2*<x_i, x_j>, so only the per-row max
off-diagonal inner product m_i is needed. Each core handles a 2048-row block
(input rolled so its rows sit at local rows 0..2047).

v2 design:
  - Phase 1: load row tiles, square-accumulate norms on ACT, Newton rsqrt,
    scale rows to unit length casting to bf16 (DVE), then transpose each
    2048-row batch with one DMA-XBAR transpose (SBUF->SBUF, 2-byte dtype)
    into xt[g]: [128, 32, 128] = (d%128, 2*tile+dhalf, row-in-tile).
  - Phase 2: bf16 matmuls (1 cycle/row vs 2 for fp32r), weights reused
    across 4 N=512 chunks per (i-tile, group, k-half).
  - Row-max reduction split across two PSUM-capable engines: DVE takes
    groups 0-3 exactly (group 0 with diagonal knocked out); ACT takes
    groups 4-7 via exp-accumulate (log-sum-exp with beta=256, shift=0.45,
    an upper bound on max with bias ~ln(k_eff)/beta ~ 1e-3 -> rel loss
    error ~1e-3, validated offline against the reference).
  - Host: combine m = max(dve_max, shift + log(sum exp)/beta), final loss.
"""

import numpy as np

import concourse.bass as bass
import concourse.mybir as mybir
import concourse.tile as tile
from concourse import bacc
from concourse import bass_utils

N = 16384
D = 256
NCORES = 8
ROWS = N // NCORES          # 2048 rows per core
ITILES = ROWS // 128        # 16 i-tiles per core
NT = N // 128               # 128 row-tiles of the full matrix
GW = 2048                   # j-group width (4 PSUM banks of fp32)
NGROUPS = N // GW           # 8 j-groups
NB = 16                     # row-tiles per batch (= one j-group)
NDVE = 4                    # groups 0..NDVE-1 reduced exactly on DVE
EPS = 1e-8
BETA = 256.0
SHIFT = 0.45

_CACHE = {}


def _build():
    f32 = mybir.dt.float32
    bf16 = mybir.dt.bfloat16
    AF = mybir.ActivationFunctionType
    ALU = mybir.AluOpType

    nc = bacc.Bacc("TRN2", target_bir_lowering=False, debug=False)
    x = nc.dram_tensor("x", [NGROUPS, NB, 128, D], f32, kind="ExternalInput").ap()
    md_out = nc.dram_tensor("md_out", [128, ITILES * NDVE], f32,
                            kind="ExternalOutput").ap()
    ms_out = nc.dram_tensor("ms_out", [128, ITILES * (NGROUPS - NDVE)], f32,
                            kind="ExternalOutput").ap()

    with tile.TileContext(nc) as tc:
        with (
            tc.tile_pool(name="singles", bufs=1) as singles,
            tc.tile_pool(name="sbig", bufs=2) as sbig_pool,
            tc.tile_pool(name="xsb", bufs=2) as xsb_pool,
            tc.tile_pool(name="xt", bufs=1) as xt_pool,
            tc.tile_pool(name="small", bufs=6) as small,
            tc.tile_pool(name="sq", bufs=4) as sq_pool,
            tc.tile_pool(name="esc", bufs=2) as esc_pool,
        ):
            # Diagonal knock-out mask: -3 on the diagonal of a 128x128 block.
            mneg = singles.tile([128, 128], f32, tag="mneg")
            nc.gpsimd.memset(mneg[:], 0.0)
            nc.gpsimd.affine_select(
                out=mneg[:],
                in_=mneg[:],
                compare_op=ALU.not_equal,
                fill=-3.0,
                base=0,
                pattern=[[-1, 128]],
                channel_multiplier=1,
            )

            ebias = singles.tile([128, 1], f32, tag="ebias")
            nc.vector.memset(ebias[:], -BETA * SHIFT)

            ss = singles.tile([128, NT], f32, tag="ss")      # norms^2, [p, tile]
            md = singles.tile([128, ITILES * NDVE], f32, tag="md")
            ms = singles.tile([128, ITILES * (NGROUPS - NDVE)], f32, tag="ms")

            # Transposed normalized matrix, one tile per j-group:
            # xt[g][dd, (2*t + k)*128 + c] = xnorm[g*2048 + t*128 + c, k*128 + dd]
            xt = [
                xt_pool.tile([128, 2 * NB * 128], bf16, tag=f"xt{g}", name=f"xt{g}")
                for g in range(NGROUPS)
            ]
            # [p, t, k, c] views for matmul operand slicing
            xtv = [
                xt[g][:].rearrange("p (t k c) -> p t k c", k=2, c=128)
                for g in range(NGROUPS)
            ]

            # ---- Phase 1: normalize rows, cast bf16, DMA-XBAR transpose ----
            for b in range(NGROUPS):
                sb = sbig_pool.tile([128, NB, D], f32, tag="sb")
                nc.sync.dma_start(
                    out=sb[:], in_=x[b].rearrange("t p d -> p t d")
                )
                for t in range(NB):
                    gt = b * NB + t
                    sq = sq_pool.tile([128, D], f32, tag="sq")
                    nc.scalar.activation(
                        sq[:], sb[:, t, :], AF.Square,
                        accum_out=ss[:, gt:gt + 1],
                    )

                # batched r = rsqrt(ss) with two Newton steps
                col = (b * NB, (b + 1) * NB)
                ssb = ss[:, col[0]:col[1]]
                sq_b = small.tile([128, NB], f32, tag="sqb")
                nc.scalar.activation(sq_b[:], ssb, AF.Sqrt)
                r = small.tile([128, NB], f32, tag="r")
                nc.vector.reciprocal(r[:], sq_b[:])
                for _ in range(2):
                    t1 = small.tile([128, NB], f32, tag="t1")
                    nc.vector.tensor_mul(t1[:], r[:], r[:])
                    nc.vector.tensor_mul(t1[:], t1[:], ssb)
                    nc.scalar.activation(t1[:], t1[:], AF.Copy, scale=-0.5, bias=1.5)
                    r2 = small.tile([128, NB], f32, tag="r")
                    nc.vector.tensor_mul(r2[:], r[:], t1[:])
                    r = r2

                xsb = xsb_pool.tile([128, NB * D], bf16, tag="xsb")
                for t in range(NB):
                    nc.vector.tensor_scalar_mul(
                        xsb[:, t * D:(t + 1) * D], sb[:, t, :], r[:, t:t + 1]
                    )
                # One XBAR transpose per 2048-row batch: [128, 4096] bf16 ->
                # [128, 32, 128] (issued on the ACT hwdge queue).
                nc.scalar.dma_start_transpose(
                    out=xt[b][:].rearrange("p (s c) -> p s c", c=128),
                    in_=xsb[:],
                )

            # ---- Phase 2: dots + row max / exp-sum ----
            with tc.tile_pool(name="dpsum", bufs=2, space="PSUM") as dpsum:
                for it in range(ITILES):
                    for g in range(NGROUPS):
                        pg = dpsum.tile([128, GW], f32, tag="pg")
                        for k in range(2):
                            lhs = xtv[0][:, it, k, :]
                            for c4 in range(4):
                                rhs = xtv[g][:, 4 * c4:4 * (c4 + 1), k, :]
                                nc.tensor.matmul(
                                    pg[:, c4 * 512:(c4 + 1) * 512],
                                    lhs, rhs,
                                    start=(k == 0), stop=(k == 1),
                                )
                        if g == 0:
                            db = 128 * it
                            nc.vector.tensor_add(
                                pg[:, db:db + 128], pg[:, db:db + 128], mneg[:]
                            )
                        if g < NDVE:
                            nc.vector.reduce_max(
                                md[:, it * NDVE + g:it * NDVE + g + 1], pg[:],
                                axis=mybir.AxisListType.X,
                            )
                        else:
                            esc = esc_pool.tile([128, GW], f32, tag="esc")
                            gs = it * (NGROUPS - NDVE) + (g - NDVE)
                            nc.scalar.activation(
                                esc[:], pg[:], AF.Exp,
                                scale=BETA, bias=ebias[:],
                                accum_out=ms[:, gs:gs + 1],
                            )

            nc.sync.dma_start(out=md_out, in_=md[:])
            nc.sync.dma_start(out=ms_out, in_=ms[:])

    nc.compile()
    return nc


def _get_nc():
    if "nc" not in _CACHE:
        _CACHE["nc"] = _build()
    return _CACHE["nc"]


def kernel(student_output: np.ndarray) -> np.ndarray:
    s = np.ascontiguousarray(np.asarray(student_output, dtype=np.float32))
    assert s.shape == (N, D)

    nc = _get_nc()
    in_maps = [
        {"x": np.ascontiguousarray(
            np.roll(s, -c * ROWS, axis=0).reshape(NGROUPS, NB, 128, D))}
        for c in range(NCORES)
    ]
    import os
    kwargs = {}
    if os.environ.get("KOLEO_TRACE"):
        kwargs = {"trace": True, "tmpdir": os.environ.get("KOLEO_TRACE_DIR") or None}
    res = bass_utils.run_bass_kernel_spmd(
        nc, in_maps, core_ids=list(range(NCORES)), **kwargs
    )
    _CACHE["last_results"] = res

    m_parts = []
    for c in range(NCORES):
        md = res.results[c]["md_out"].astype(np.float64)   # [128, 16*NDVE]
        ms = res.results[c]["ms_out"].astype(np.float64)   # [128, 16*(8-NDVE)]
        md = md.reshape(128, ITILES, NDVE)
        ms = ms.reshape(128, ITILES, NGROUPS - NDVE)
        m_dve = md.max(axis=2)                             # [128, it]
        s_sum = ms.sum(axis=2)                             # [128, it]
        with np.errstate(divide="ignore"):
            m_lse = SHIFT + np.log(s_sum) / BETA
        m_loc = np.maximum(m_dve, m_lse)                   # [p, it]
        m_parts.append(m_loc.T.reshape(ROWS))              # local row = it*128+p
    m = np.concatenate(m_parts)

    d2 = np.maximum(2.0 - 2.0 * m, 0.0)
    loss = -np.mean(np.log(np.sqrt(d2) + EPS))
    return np.array(loss, dtype=np.float32)
